# revision 8
# baseline (speedup 1.0000x reference)
"""Trainium2 Bass kernel for nn_AutoCorr2D.

Computation (per sample):
  f   = conv3x3(x, w_ext, pad=1) + b_ext            # [CC=128, 64, 64]
  corr[c,i,j,k] = f[c,i,j] * fpad[c, i+u-2, j+v-2]  # 5x5 window products
  out[o,i,j]    = sum_{c,k} w_reg[o,c,k] * corr[c,i,j,k] + b_reg[o]

Sharding: data-parallel over batch B=8 across 8 NeuronCores (one sample per
core); conv weights replicated.

Per-core implementation:
  stage 1: implicit GEMM over (cin_tile, 3x3 tap): 18 accumulating matmuls
           per 512-pixel chunk, reading shifted views of a zero-padded x
           buffer; bias folded into the PSUM->SBUF copy (ScalarE Identity).
  stage 2: product symmetry: P_{a,b}[y,x] = fext[y,x]*fext[y+a,x+b] serves
           both tap (a,b) (read at [i,j]) and tap (-a,-b) (read at
           [i-a,j-b]), so only 13 of 25 product maps are computed
           (ScalarE Square for (0,0), VectorE / GpSimd for the rest).
           Then 25 accumulating matmuls (K=128 channels per tap) per chunk
           into PSUM[64,512], bias-copied to SBUF and DMA'd out.
  Matmuls run as float32r (full streaming rate for N>=256).
"""

import numpy as np

from concourse import bacc, mybir, tile
from concourse.bass_utils import run_bass_kernel_spmd

B, CIN, H, W = 8, 256, 64, 64
CC, COUT = 128, 64
HW = H * W
NCORES = 8

NCHUNK = 8          # pixel chunks per image
CROWS = H // NCHUNK  # rows per chunk (8) -> N = 512 pixels
NPX = CROWS * W      # 512

XP = W + 2           # xpad cols (pad=1)
XR = H + 2           # xpad rows
FP = W + 4           # fpad cols (pad=2)
FR = H + 4           # fpad rows
FTAIL = 72           # guard tail so shifted product reads stay in-bounds
PROWS = CROWS + 2    # product-map rows per chunk (apron for reflected taps)

# The 13 "upper half" taps; (a,b) also serves tap (-a,-b) via a shifted read.
SYM = [(0, 0), (0, 1), (0, 2),
       (1, -2), (1, -1), (1, 0), (1, 1), (1, 2),
       (2, -2), (2, -1), (2, 0), (2, 1), (2, 2)]
# Product-map engine split: index into SYM (0 is always ScalarE Square).
POOL_KS = {3, 5, 8, 10}  # computed on GpSimd; rest on VectorE

F32 = mybir.dt.float32
F32R = mybir.dt.float32r
AF = mybir.ActivationFunctionType


def build_body(nc, tc, x, wext, wreg, bext, breg, out):
    with (
        tc.tile_pool(name="const", bufs=1) as constp,
        tc.tile_pool(name="xpadp", bufs=1) as xpadp,
        tc.tile_pool(name="fpadp", bufs=1) as fpadp,
        tc.tile_pool(name="prodp", bufs=2) as prodp,
        tc.tile_pool(name="outp", bufs=3) as outp,
        tc.tile_pool(name="ps1", bufs=2, space="PSUM") as ps1,
        tc.tile_pool(name="ps2", bufs=3, space="PSUM") as ps2,
    ):
        # weights cast f32 -> f32r in the DMA (SWDGE cast, gpsimd-initiated)
        wext_sb = constp.tile([128, 18 * 128], F32R, name="wext_sb")
        nc.gpsimd.dma_start(out=wext_sb, in_=wext)
        wreg_sb = constp.tile([128, 25 * 64], F32R, name="wreg_sb")
        nc.gpsimd.dma_start(out=wreg_sb, in_=wreg)
        bext_sb = constp.tile([128, 1], F32, name="bext_sb")
        nc.sync.dma_start(out=bext_sb, in_=bext)
        breg_sb = constp.tile([64, 1], F32, name="breg_sb")
        nc.sync.dma_start(out=breg_sb, in_=breg)

        # ---- padded input (pad=1), one tile per 128-channel cin group ----
        xpads = []
        for t in range(2):
            xp = xpadp.tile([128, XR * XP], F32R, name=f"xpad{t}", tag=f"xpad{t}")
            xr = xp.rearrange("p (r c) -> p r c", c=XP)
            # borders: zero bit-pattern via uint32 view (memset can't target f32r)
            xri = xp.bitcast(mybir.dt.uint32).rearrange("p (r c) -> p r c", c=XP)
            nc.vector.memset(xri[:, 0, :], 0)
            nc.vector.memset(xri[:, XR - 1, :], 0)
            nc.vector.memset(xri[:, 1:XR - 1, 0], 0)
            nc.vector.memset(xri[:, 1:XR - 1, XP - 1], 0)
            xpads.append(xr)
        # interior DMA in row bands so chunk 0 can start early
        NBAND = 4
        BROWS = H // NBAND
        for band in range(NBAND):
            for t in range(2):
                r0 = band * BROWS
                dst = xpads[t][:, 1 + r0:1 + r0 + BROWS, 1:1 + W]
                src = x[t * 128:(t + 1) * 128, r0 * W:(r0 + BROWS) * W]
                src = src.rearrange("p (r c) -> p r c", c=W)
                nc.gpsimd.dma_start(out=dst, in_=src)

        # ---- padded features (pad=2) + guard tail ----
        fpad = fpadp.tile([128, FR * FP + FTAIL], F32, name="fpad")
        fr = fpad[:, :FR * FP].rearrange("p (r c) -> p r c", c=FP)
        nc.vector.memset(fpad[:, 0:2 * FP], 0.0)
        nc.vector.memset(fpad[:, (FR - 2) * FP:FR * FP + FTAIL], 0.0)
        nc.vector.memset(fr[:, 2:FR - 2, 0:2], 0.0)
        nc.vector.memset(fr[:, 2:FR - 2, FP - 2:FP], 0.0)

        # ---- stage 1: f = conv3x3(x) + b_ext ----
        for i in range(NCHUNK):
            psum1 = ps1.tile([128, NPX], F32, name="psum1", tag="psum1")
            k = 0
            for t in range(2):
                for du in range(3):
                    for dv in range(3):
                        rhs = xpads[t][:, i * CROWS + du:i * CROWS + du + CROWS,
                                       dv:dv + W]
                        lhsT = wext_sb[:, ((du * 3 + dv) * 2 + t) * 128:
                                       ((du * 3 + dv) * 2 + t + 1) * 128]
                        nc.tensor.matmul(psum1, lhsT, rhs,
                                         start=(k == 0), stop=(k == 17))
                        k += 1
            dst = fr[:, i * CROWS + 2:i * CROWS + 2 + CROWS, 2:2 + W]
            nc.scalar.activation(dst,
                                 psum1.rearrange("p (r c) -> p r c", c=W),
                                 AF.Identity, bias=bext_sb, scale=1.0)

        # ---- stage 2: products + regressor GEMM ----
        for i in range(NCHUNK):
            base = i * CROWS * FP
            ptiles = []
            for k, (a, b) in enumerate(SYM):
                pt = prodp.tile([128, PROWS * FP], F32R, name=f"prod{k}",
                                tag=f"prod{k}")
                in0 = fpad[:, base:base + PROWS * FP]
                in1 = fpad[:, base + a * FP + b:base + a * FP + b + PROWS * FP]
                if k == 0:
                    nc.scalar.activation(pt, in0, AF.Square)
                elif k in POOL_KS:
                    nc.gpsimd.tensor_mul(pt, in0, in1)
                else:
                    nc.vector.tensor_mul(pt, in0, in1)
                ptiles.append(pt)

            psum2 = ps2.tile([COUT, NPX], F32, name="psum2", tag="psum2")
            mm = 0
            for k, (a, b) in enumerate(SYM):
                pr = ptiles[k].rearrange("p (r c) -> p r c", c=FP)
                taps = [(a, b)] if (a, b) == (0, 0) else [(a, b), (-a, -b)]
                for (p, q) in taps:
                    if (p, q) == (a, b):
                        rhs = pr[:, 2:2 + CROWS, 2:2 + W]
                    else:
                        rhs = pr[:, 2 - a:2 - a + CROWS, 2 - b:2 - b + W]
                    tidx = (p + 2) * 5 + (q + 2)
                    lhsT = wreg_sb[:, tidx * 64:(tidx + 1) * 64]
                    nc.tensor.matmul(psum2, lhsT, rhs,
                                     start=(mm == 0), stop=(mm == 24))
                    mm += 1

            outt = outp.tile([COUT, NPX], F32, name="outsb", tag="outsb")
            nc.scalar.activation(outt, psum2, AF.Identity, bias=breg_sb,
                                 scale=1.0)
            nc.sync.dma_start(out=out[:, i * NPX:(i + 1) * NPX], in_=outt)


def build_nc():
    nc = bacc.Bacc("TRN2", target_bir_lowering=False, debug=False,
                   num_devices=NCORES)
    x = nc.dram_tensor("x", [CIN, HW], F32, kind="ExternalInput").ap()
    wext = nc.dram_tensor("wext", [128, 18 * 128], F32,
                          kind="ExternalInput").ap()
    wreg = nc.dram_tensor("wreg", [128, 25 * 64], F32,
                          kind="ExternalInput").ap()
    bext = nc.dram_tensor("bext", [128, 1], F32, kind="ExternalInput").ap()
    breg = nc.dram_tensor("breg", [64, 1], F32, kind="ExternalInput").ap()
    out = nc.dram_tensor("out", [COUT, HW], F32, kind="ExternalOutput").ap()
    with tile.TileContext(nc) as tc:
        build_body(nc, tc, x, wext, wreg, bext, breg, out)
    nc.compile()
    return nc


def prep_in_maps(x, w_ext, b_ext, w_reg, b_reg):
    x = np.ascontiguousarray(np.asarray(x, dtype=np.float32))
    w_ext = np.asarray(w_ext, dtype=np.float32)
    w_reg = np.asarray(w_reg, dtype=np.float32)
    b_ext = np.asarray(b_ext, dtype=np.float32)
    b_reg = np.asarray(b_reg, dtype=np.float32)

    # lhsT layouts: wext [cin(128-part), (tap,cintile)*cc], wreg [cc, tap*cout]
    w1 = np.transpose(w_ext, (1, 2, 3, 0))          # [CIN, 3, 3, CC]
    wext_p = np.zeros((128, 18, 128), np.float32)
    for du in range(3):
        for dv in range(3):
            for t in range(2):
                wext_p[:, (du * 3 + dv) * 2 + t, :] = \
                    w1[t * 128:(t + 1) * 128, du, dv, :]
    wext_p = np.ascontiguousarray(wext_p.reshape(128, 18 * 128))
    w2 = np.transpose(w_reg, (1, 2, 3, 0))          # [CC, 5, 5, COUT]
    wreg_p = np.ascontiguousarray(w2.reshape(128, 25 * 64))
    bext_p = np.ascontiguousarray(b_ext.reshape(128, 1))
    breg_p = np.ascontiguousarray(b_reg.reshape(64, 1))

    return [{
        "x": np.ascontiguousarray(x[b].reshape(CIN, HW)),
        "wext": wext_p,
        "wreg": wreg_p,
        "bext": bext_p,
        "breg": breg_p,
    } for b in range(B)]


_NC_CACHE = None


def kernel(x, w_ext, b_ext, w_reg, b_reg):
    global _NC_CACHE
    if _NC_CACHE is None:
        _NC_CACHE = build_nc()
    nc = _NC_CACHE
    in_maps = prep_in_maps(x, w_ext, b_ext, w_reg, b_reg)
    res = run_bass_kernel_spmd(nc, in_maps, list(range(NCORES)))
    return np.stack([res.results[b]["out"].reshape(COUT, H, W)
                     for b in range(B)], axis=0)


# revision 12
# speedup vs baseline: 1.1877x; 1.1877x over previous
"""Trainium2 Bass kernel for nn_AutoCorr2D.

Computation (per sample):
  f   = conv3x3(x, w_ext, pad=1) + b_ext            # [CC=128, 64, 64]
  corr[c,i,j,k] = f[c,i,j] * fpad[c, i+u-2, j+v-2]  # 5x5 window products
  out[o,i,j]    = sum_{c,k} w_reg[o,c,k] * corr[c,i,j,k] + b_reg[o]

Sharding: data-parallel over batch B=8 across 8 NeuronCores (one sample per
core); conv weights replicated.

Per-core implementation:
  stage 1: implicit GEMM over (cin_tile, 3x3 tap): 18 accumulating matmuls
           per 512-pixel chunk, reading shifted views of a zero-padded x
           buffer; bias folded into the PSUM->SBUF copy (ScalarE Identity).
  stage 2: product symmetry: P_{a,b}[y,x] = fext[y,x]*fext[y+a,x+b] serves
           both tap (a,b) (read at [i,j]) and tap (-a,-b) (read at
           [i-a,j-b]), so only 13 of 25 product maps are computed
           (ScalarE Square for (0,0), VectorE / GpSimd for the rest).
           Then 25 accumulating matmuls (K=128 channels per tap) per chunk
           into PSUM[64,512], bias-copied to SBUF and DMA'd out.
  Matmuls run as float32r (full streaming rate for N>=256).
"""

import numpy as np

from concourse import bacc, mybir, tile
from concourse.bass_utils import run_bass_kernel_spmd

B, CIN, H, W = 8, 256, 64, 64
CC, COUT = 128, 64
HW = H * W
NCORES = 8

NCHUNK = 8          # pixel chunks per image
CROWS = H // NCHUNK  # rows per chunk (8) -> N = 512 pixels
NPX = CROWS * W      # 512

XP = W + 2           # xpad cols (pad=1)
XR = H + 2           # xpad rows
FP = W + 4           # fpad cols (pad=2)
FR = H + 4           # fpad rows
FTAIL = 72           # guard tail so shifted product reads stay in-bounds
PROWS = CROWS + 2    # product-map rows per chunk (apron for reflected taps)

# The 13 "upper half" taps; (a,b) also serves tap (-a,-b) via a shifted read.
SYM = [(0, 0), (0, 1), (0, 2),
       (1, -2), (1, -1), (1, 0), (1, 1), (1, 2),
       (2, -2), (2, -1), (2, 0), (2, 1), (2, 2)]
# Products: SYM[0] = f^2 on ScalarE (Square); the other 12 on VectorE.
# GpSimd shares the DVE SBUF port — concurrent tensor_tensor there halves
# both engines' rates (measured), so it gets no product work.

F32 = mybir.dt.float32
F32R = mybir.dt.float32r
AF = mybir.ActivationFunctionType


def build_body(nc, tc, x, wext, wreg, bext, breg, out):
    with (
        tc.tile_pool(name="const", bufs=1) as constp,
        tc.tile_pool(name="xpadp", bufs=1) as xpadp,
        tc.tile_pool(name="fpadp", bufs=1) as fpadp,
        tc.tile_pool(name="prodp", bufs=3) as prodp,
        tc.tile_pool(name="outp", bufs=3) as outp,
        tc.tile_pool(name="ps1", bufs=2, space="PSUM") as ps1,
        tc.tile_pool(name="ps2", bufs=4, space="PSUM") as ps2,
        tc.tile_pool(name="warmp", bufs=1, space="PSUM") as warmp,
    ):
        # weights cast f32 -> f32r in the DMA (SWDGE cast, gpsimd-initiated)
        wext_sb = constp.tile([128, 18 * 128], F32R, name="wext_sb")
        nc.gpsimd.dma_start(out=wext_sb, in_=wext)
        wreg_sb = constp.tile([128, 25 * 64], F32R, name="wreg_sb")
        nc.gpsimd.dma_start(out=wreg_sb, in_=wreg)
        bext_sb = constp.tile([128, 1], F32, name="bext_sb")
        nc.sync.dma_start(out=bext_sb, in_=bext)
        breg_sb = constp.tile([64, 1], F32, name="breg_sb")
        nc.sync.dma_start(out=breg_sb, in_=breg)

        # PE warm-up: dummy matmuls during the DMA prologue so the HAM clock
        # gate is released (~3.4us of activity) before real matmuls start.
        wsc = constp.tile([128, NPX], F32, name="wsc")
        nc.vector.memset(wsc, 0.0)
        wsc_r = constp.tile([128, NPX], F32R, name="wsc_r")
        nc.scalar.activation(wsc_r, wsc, AF.Copy)
        wpsum = warmp.tile([128, NPX], F32, name="wpsum")
        for i in range(12):
            nc.tensor.matmul(wpsum, wsc_r[:, :128], wsc_r,
                             start=(i == 0), stop=(i == 11))

        # ---- padded input (pad=1), one tile per 128-channel cin group ----
        xpads = []
        for t in range(2):
            xp = xpadp.tile([128, XR * XP], F32R, name=f"xpad{t}", tag=f"xpad{t}")
            xr = xp.rearrange("p (r c) -> p r c", c=XP)
            # borders: zero bit-pattern via uint32 view (memset can't target f32r)
            xri = xp.bitcast(mybir.dt.uint32).rearrange("p (r c) -> p r c", c=XP)
            nc.vector.memset(xri[:, 0, :], 0)
            nc.vector.memset(xri[:, XR - 1, :], 0)
            nc.vector.memset(xri[:, 1:XR - 1, 0], 0)
            nc.vector.memset(xri[:, 1:XR - 1, XP - 1], 0)
            xpads.append(xr)
        # interior DMA in row bands so chunk 0 can start early
        NBAND = 4
        BROWS = H // NBAND
        for band in range(NBAND):
            for t in range(2):
                r0 = band * BROWS
                dst = xpads[t][:, 1 + r0:1 + r0 + BROWS, 1:1 + W]
                src = x[t * 128:(t + 1) * 128, r0 * W:(r0 + BROWS) * W]
                src = src.rearrange("p (r c) -> p r c", c=W)
                nc.gpsimd.dma_start(out=dst, in_=src)

        # ---- padded features (pad=2) + guard tail ----
        fpad = fpadp.tile([128, FR * FP + FTAIL], F32, name="fpad")
        fr = fpad[:, :FR * FP].rearrange("p (r c) -> p r c", c=FP)
        nc.vector.memset(fpad[:, 0:2 * FP], 0.0)
        nc.vector.memset(fpad[:, (FR - 2) * FP:FR * FP + FTAIL], 0.0)
        nc.vector.memset(fr[:, 2:FR - 2, 0:2], 0.0)
        nc.vector.memset(fr[:, 2:FR - 2, FP - 2:FP], 0.0)

        # ---- stage 1: f = conv3x3(x) + b_ext ----
        for i in range(NCHUNK):
            psum1 = ps1.tile([128, NPX], F32, name="psum1", tag="psum1")
            k = 0
            for t in range(2):
                for du in range(3):
                    for dv in range(3):
                        rhs = xpads[t][:, i * CROWS + du:i * CROWS + du + CROWS,
                                       dv:dv + W]
                        lhsT = wext_sb[:, ((du * 3 + dv) * 2 + t) * 128:
                                       ((du * 3 + dv) * 2 + t + 1) * 128]
                        nc.tensor.matmul(psum1, lhsT, rhs,
                                         start=(k == 0), stop=(k == 17))
                        k += 1
            dst = fr[:, i * CROWS + 2:i * CROWS + 2 + CROWS, 2:2 + W]
            nc.scalar.activation(dst,
                                 psum1.rearrange("p (r c) -> p r c", c=W),
                                 AF.Identity, bias=bext_sb, scale=1.0)

        # ---- stage 2: products + regressor GEMM ----
        for i in range(NCHUNK):
            base = i * CROWS * FP
            ptiles = []
            for k, (a, b) in enumerate(SYM):
                pt = prodp.tile([128, PROWS * FP], F32R, name=f"prod{k}",
                                tag=f"prod{k}")
                in0 = fpad[:, base:base + PROWS * FP]
                in1 = fpad[:, base + a * FP + b:base + a * FP + b + PROWS * FP]
                if k == 0:
                    nc.scalar.activation(pt, in0, AF.Square)
                else:
                    nc.vector.tensor_mul(pt, in0, in1)
                ptiles.append(pt)

            psum2 = ps2.tile([COUT, NPX], F32, name="psum2", tag="psum2")
            mm = 0
            for k, (a, b) in enumerate(SYM):
                pr = ptiles[k].rearrange("p (r c) -> p r c", c=FP)
                taps = [(a, b)] if (a, b) == (0, 0) else [(a, b), (-a, -b)]
                for (p, q) in taps:
                    if (p, q) == (a, b):
                        rhs = pr[:, 2:2 + CROWS, 2:2 + W]
                    else:
                        rhs = pr[:, 2 - a:2 - a + CROWS, 2 - b:2 - b + W]
                    tidx = (p + 2) * 5 + (q + 2)
                    lhsT = wreg_sb[:, tidx * 64:(tidx + 1) * 64]
                    nc.tensor.matmul(psum2, lhsT, rhs,
                                     start=(mm == 0), stop=(mm == 24))
                    mm += 1

            outt = outp.tile([COUT, NPX], F32, name="outsb", tag="outsb")
            nc.scalar.activation(outt, psum2, AF.Identity, bias=breg_sb,
                                 scale=1.0)
            nc.sync.dma_start(out=out[:, i * NPX:(i + 1) * NPX], in_=outt)


def build_nc():
    nc = bacc.Bacc("TRN2", target_bir_lowering=False, debug=False,
                   num_devices=NCORES)
    x = nc.dram_tensor("x", [CIN, HW], F32, kind="ExternalInput").ap()
    wext = nc.dram_tensor("wext", [128, 18 * 128], F32,
                          kind="ExternalInput").ap()
    wreg = nc.dram_tensor("wreg", [128, 25 * 64], F32,
                          kind="ExternalInput").ap()
    bext = nc.dram_tensor("bext", [128, 1], F32, kind="ExternalInput").ap()
    breg = nc.dram_tensor("breg", [64, 1], F32, kind="ExternalInput").ap()
    out = nc.dram_tensor("out", [COUT, HW], F32, kind="ExternalOutput").ap()
    with tile.TileContext(nc) as tc:
        build_body(nc, tc, x, wext, wreg, bext, breg, out)
    nc.compile()
    return nc


def prep_in_maps(x, w_ext, b_ext, w_reg, b_reg):
    x = np.ascontiguousarray(np.asarray(x, dtype=np.float32))
    w_ext = np.asarray(w_ext, dtype=np.float32)
    w_reg = np.asarray(w_reg, dtype=np.float32)
    b_ext = np.asarray(b_ext, dtype=np.float32)
    b_reg = np.asarray(b_reg, dtype=np.float32)

    # lhsT layouts: wext [cin(128-part), (tap,cintile)*cc], wreg [cc, tap*cout]
    w1 = np.transpose(w_ext, (1, 2, 3, 0))          # [CIN, 3, 3, CC]
    wext_p = np.zeros((128, 18, 128), np.float32)
    for du in range(3):
        for dv in range(3):
            for t in range(2):
                wext_p[:, (du * 3 + dv) * 2 + t, :] = \
                    w1[t * 128:(t + 1) * 128, du, dv, :]
    wext_p = np.ascontiguousarray(wext_p.reshape(128, 18 * 128))
    w2 = np.transpose(w_reg, (1, 2, 3, 0))          # [CC, 5, 5, COUT]
    wreg_p = np.ascontiguousarray(w2.reshape(128, 25 * 64))
    bext_p = np.ascontiguousarray(b_ext.reshape(128, 1))
    breg_p = np.ascontiguousarray(b_reg.reshape(64, 1))

    return [{
        "x": np.ascontiguousarray(x[b].reshape(CIN, HW)),
        "wext": wext_p,
        "wreg": wreg_p,
        "bext": bext_p,
        "breg": breg_p,
    } for b in range(B)]


_NC_CACHE = None


def kernel(x, w_ext, b_ext, w_reg, b_reg):
    global _NC_CACHE
    if _NC_CACHE is None:
        _NC_CACHE = build_nc()
    nc = _NC_CACHE
    in_maps = prep_in_maps(x, w_ext, b_ext, w_reg, b_reg)
    res = run_bass_kernel_spmd(nc, in_maps, list(range(NCORES)))
    return np.stack([res.results[b]["out"].reshape(COUT, H, W)
                     for b in range(B)], axis=0)
